# revision 1
# baseline (speedup 1.0000x reference)
"""DiT forward pass on 8 Trainium2 NeuronCores (Bass/Tile).

Model: L=4 layers, D=768, H=12 heads (hd=64), cond=128, V=32000, B=2, S=1024.

Sharding: 8 cores = 2 batch groups x 4 token shards (256 tokens/core).
Per layer the only collective is an AllGather of (k^T, v) within each
4-core batch group; attention is query-sharded.  The final unembedding is
vocab-sharded (8000 cols/core) after an AllGather of the final modulated
activations.  All matmuls run in float32r (full PE rate).

Notes on scale folding: 1/sqrt(hd) is folded into the q rows of qkv_w on
the host.  Softmax runs without max-subtraction (logits are < 10 here).
The AllGathered v carries an extra all-ones column per head (v_aug), so
the attention AV matmul also produces the softmax denominators.
"""
import sys
import math

sys.path.insert(0, "/opt/trn_rl_repo")

import numpy as np
import concourse.bass as bass
import concourse.mybir as mybir
import concourse.tile as tile
from concourse import bacc
from concourse.bass_utils import run_bass_kernel_spmd
from concourse.masks import make_identity

F32 = mybir.dt.float32
F32R = mybir.dt.float32r
I32 = mybir.dt.int32
AF = mybir.ActivationFunctionType
OP = mybir.AluOpType

L, D, H, HD, COND, FREQ, V, B, S = 4, 768, 12, 64, 128, 256, 32000, 2, 1024
MLP = 4 * D
EPS = 1e-5
P = 128
NCORES = 8
TPD = 4              # token-parallel degree (group size)
T = S // TPD         # tokens per core (256)
TT = T // P          # token tiles per core (2)
DC = D // P          # feature chunks (6)
VSH = V // TPD       # vocab shard (8000)
GROUPS = [[0, 1, 2, 3], [4, 5, 6, 7]]
C6 = 6 * D           # 4608
UN = 16              # unembed n-tiles
UNW = VSH // UN      # 500
KV1 = DC * P * T     # bytes offset (elems) of v in the kv AG block

DEBUG = False        # set True (before _get_nc) to add intermediate dumps
CC_EMU = False       # replace collectives with local DMA copies (for TimelineSim)


def _build():
    nc = bacc.Bacc("TRN2", target_bir_lowering=False, debug=False,
                   enable_asserts=False, num_devices=NCORES)

    def inp(name, shape, dt=F32R):
        return nc.dram_tensor(name, shape, dt, kind="ExternalInput")

    idx_d = inp("idx", [T, 1], I32)
    emb_d = inp("embtab", [V, D])
    sig_d = inp("sig11", [1, 1], F32)
    freqsT_d = inp("freqsT", [P, 1], F32)
    tw1T_d = inp("tw1T", [2, P, P])
    tb1_d = inp("tb1", [1, P])
    tw2T_d = inp("tw2T", [P, P])
    tb2_d = inp("tb2", [1, P])
    adawT_d = inp("adawT", [P, L * C6 + 2 * D])
    badd_d = inp("badd", [1, L * C6 + 2 * D])
    n1w_d = inp("n1w", [L, 1, D], F32)
    n2w_d = inp("n2w", [L, 1, D], F32)
    nfw_d = inp("nfw", [1, D], F32)
    cosn_d = inp("cosn", [T, D], F32)
    sinn_d = inp("sinn", [T, D], F32)
    wqkv_d = inp("wqkv", [L, 6, DC, P, 384])
    wao_d = inp("wao", [L, 2, DC, P, 384])
    w1_d = inp("w1", [L, 24, P, DC, P])
    b1T_d = inp("b1T", [L, P, 24], F32)
    w2_d = inp("w2", [L, MLP, D])
    b2_d = inp("b2r", [L, 1, D])
    linwT_d = inp("linwT", [D, VSH])
    linb_d = inp("linb", [1, VSH], F32)
    out_d = nc.dram_tensor("out", [T * TPD, VSH], F32, kind="ExternalOutput")

    dbg_specs = {
        "dbg_x0": [TT, P, D], "dbg_c": [P, 2], "dbg_bc0": [P, 3 * D],
        "dbg_xm0": [TT, P, D], "dbg_qkv0": [TT, P, 3 * D],
        "dbg_kTl0": [P, DC * T], "dbg_agKT0": [P, TPD * DC * T],
        "dbg_vaug0": [P, 2 * TPD * H * (HD + 1)], "dbg_qT0": [P, DC * T],
        "dbg_exp0": [P, T], "dbg_onorm0": [P, DC * T],
        "dbg_xa0": [TT, P, D], "dbg_x1": [TT, P, D],
        "dbg_xmf": [TT, P, D], "dbg_agX": [P, TPD * DC * T],
    }
    dbg = {}
    if DEBUG:
        for k, shp in dbg_specs.items():
            dbg[k] = nc.dram_tensor(k, shp, F32, kind="ExternalOutput")

    def ddump(name, ap, sub=None):
        if not DEBUG:
            return
        tgt = dbg[name]
        nc.gpsimd.dma_start(tgt[sub] if sub is not None else tgt[:], ap)

    with tile.TileContext(nc) as tc:
        with (
            tc.tile_pool(name="pers", bufs=1) as pers,
            tc.tile_pool(name="work", bufs=1) as work,
            tc.tile_pool(name="wts", bufs=4) as wts,
            tc.tile_pool(name="ps", bufs=3, space="PSUM") as ps,
            tc.tile_pool(name="dram", bufs=2, space="DRAM") as dram,
        ):
            # ---------- persistent setup ----------
            # (memset on f32r is not a legal ISA op: build consts in f32, copy)
            identF = pers.tile([P, P], F32)
            make_identity(nc, identF[:])
            identR = pers.tile([P, P], F32R)
            nc.vector.tensor_copy(identR[:], identF[:])
            epsB = pers.tile([P, 1], F32)
            nc.gpsimd.memset(epsB[:], EPS)
            halfpi = pers.tile([P, 1], F32)
            nc.gpsimd.memset(halfpi[:], float(np.pi / 2))
            onesF = pers.tile([P, P], F32)
            nc.gpsimd.memset(onesF[:], 1.0)
            ones1r = pers.tile([1, P], F32R)
            nc.vector.tensor_copy(ones1r[:], onesF[0:1, :])
            ones12 = pers.tile([1, 2], F32R)
            nc.vector.tensor_copy(ones12[:], onesF[0:1, 0:2])

            cos_sb = pers.tile([P, TT, D], F32)
            sin_sb = pers.tile([P, TT, D], F32)
            nc.sync.dma_start(cos_sb[:], cosn_d[:, :].rearrange("(t p) c -> p t c", p=P))
            nc.sync.dma_start(sin_sb[:], sinn_d[:, :].rearrange("(t p) c -> p t c", p=P))
            b1T = pers.tile([P, L, 24], F32)
            nc.sync.dma_start(b1T[:], b1T_d[:, :, :].rearrange("l p c -> p l c"))

            # embedding gather
            idx_sb = pers.tile([P, TT], I32)
            nc.sync.dma_start(idx_sb[:], idx_d[:, :].rearrange("(t p) o -> p (t o)", p=P))
            x = pers.tile([P, TT, D], F32R)
            for t in range(TT):
                nc.gpsimd.indirect_dma_start(
                    out=x[:, t, :], out_offset=None, in_=emb_d[:, :],
                    in_offset=bass.IndirectOffsetOnAxis(ap=idx_sb[:, t:t + 1], axis=0))
                ddump("dbg_x0", x[:, t, :], t)

            # ---------- timestep embedder -> cT_rep ----------
            sig11 = pers.tile([1, 1], F32)
            nc.sync.dma_start(sig11[:], sig_d[:, :])
            sigB = pers.tile([P, 1], F32)
            nc.gpsimd.partition_broadcast(sigB[:], sig11[:])
            freqsT = pers.tile([P, 1], F32)
            nc.sync.dma_start(freqsT[:], freqsT_d[:, :])
            sigfT = pers.tile([P, 1], F32)
            nc.vector.tensor_scalar_mul(sigfT[:], freqsT[:], sigB[:])
            # fp32r matmuls need even moving-dim: duplicate every column
            tembT = pers.tile([P, 2, 2], F32R)
            nc.scalar.activation(tembT[:, 0, :], sigfT[:, 0:1].to_broadcast([P, 2]),
                                 AF.Sin, bias=halfpi[:])
            nc.scalar.activation(tembT[:, 1, :], sigfT[:, 0:1].to_broadcast([P, 2]),
                                 AF.Sin)

            tw1Tf = pers.tile([P, 2 * P], F32R)
            nc.sync.dma_start(tw1Tf[:, 0:P], tw1T_d[0, :, :])
            nc.sync.dma_start(tw1Tf[:, P:2 * P], tw1T_d[1, :, :])
            tb1 = pers.tile([1, P], F32R)
            nc.sync.dma_start(tb1[:], tb1_d[:, :])
            tw2T = pers.tile([P, P], F32R)
            nc.sync.dma_start(tw2T[:], tw2T_d[:, :])
            tb2 = pers.tile([1, P], F32R)
            nc.sync.dma_start(tb2[:], tb2_d[:, :])

            t1p = ps.tile([P, 2], F32, tag="tr", bufs=2)
            nc.tensor.matmul(t1p[:], tw1Tf[:, 0:P], tembT[:, 0, :], start=True, stop=False)
            nc.tensor.matmul(t1p[:], tw1Tf[:, P:2 * P], tembT[:, 1, :], start=False, stop=False)
            nc.tensor.matmul(t1p[:], tb1[:], ones12[:], start=False, stop=True)
            t1T = pers.tile([P, 2], F32R)
            nc.scalar.activation(t1T[:], t1p[:], AF.Silu)
            t2p = ps.tile([P, 2], F32, tag="tr", bufs=2)
            nc.tensor.matmul(t2p[:], tw2T[:], t1T[:], start=True, stop=False)
            nc.tensor.matmul(t2p[:], tb2[:], ones12[:], start=False, stop=True)
            cT = pers.tile([P, 2], F32R)
            nc.scalar.activation(cT[:], t2p[:], AF.Silu)
            cT_rep = pers.tile([P, P], F32R)
            nc.vector.tensor_copy(cT_rep[:], cT[:, 0:1].to_broadcast([P, P]))
            ddump("dbg_c", cT[:])

            # ---------- helpers ----------
            ev_state = [0]

            def ev_copy(out, in_):
                ev_state[0] ^= 1
                if ev_state[0]:
                    nc.scalar.copy(out, in_)
                else:
                    nc.vector.tensor_copy(out, in_)

            def emit_bc(off, size):
                """broadcasted (c6 @ adaW + bias) block [P, size]"""
                bc = work.tile([P, size], F32, tag="bc", padded_shape=[P, 3 * D])
                n0 = 0
                while n0 < size:
                    w = min(512, size - n0)
                    o = off + n0
                    pt = ps.tile([P, 512], F32, tag="mm", bufs=3)
                    ada = wts.tile([P, 512], F32R, tag="ada", bufs=3)
                    nc.sync.dma_start(ada[:, 0:w], adawT_d[:, o:o + w])
                    bslc = work.tile([1, 512], F32R, tag="badd", bufs=2)
                    nc.sync.dma_start(bslc[:, 0:w], badd_d[:, o:o + w])
                    nc.tensor.matmul(pt[:, 0:w], cT_rep[:], ada[:, 0:w],
                                     start=True, stop=False)
                    nc.tensor.matmul(pt[:, 0:w], ones1r[:], bslc[:, 0:w],
                                     start=False, stop=True)
                    ev_copy(bc[:, n0:n0 + w], pt[:, 0:w])
                    n0 += w
                return bc

            def ln_mod(t, weffB, shB, xm_out):
                """xm_out[:, t, :] = LN(x[:, t, :]) * weffB + shB"""
                st = work.tile([P, 2, 6], F32, tag="lnst")
                nc.vector.bn_stats(st[:, 0, :], x[:, t, 0:D // 2])
                nc.vector.bn_stats(st[:, 1, :], x[:, t, D // 2:D])
                agg = work.tile([P, 2], F32, tag="lnagg")
                nc.vector.bn_aggr(agg[:], st[:])
                sq = work.tile([P, 1], F32, tag="lnsq")
                nc.scalar.activation(sq[:], agg[:, 1:2], AF.Sqrt, bias=epsB[:])
                rs = work.tile([P, 1], F32, tag="lnrs")
                nc.vector.reciprocal(rs[:], sq[:])
                tln = work.tile([P, D], F32, tag="tln")
                nc.vector.tensor_scalar(tln[:], x[:, t, :], agg[:, 0:1], rs[:],
                                        OP.subtract, OP.mult)
                tmp = work.tile([P, D], F32, tag="lntmp")
                nc.vector.tensor_tensor(tmp[:], tln[:], weffB[:], op=OP.mult)
                nc.vector.tensor_tensor(xm_out[:, t, :], tmp[:], shB, op=OP.add)

            def pbcast_row(dram_ap):
                row = work.tile([1, D], F32, tag="nwrow")
                nc.sync.dma_start(row[:], dram_ap)
                b = work.tile([P, D], F32, tag="nwb")
                nc.gpsimd.partition_broadcast(b[:], row[:])
                return b

            def weff(bc, sc_off, nw_b):
                w = work.tile([P, D], F32, tag="weff")
                nc.vector.tensor_tensor(w[:], bc[:, sc_off:sc_off + D], nw_b[:], op=OP.mult)
                return w

            def transpose_into(dst_ap, src_ap):
                tp = ps.tile([P, P], F32R, tag="tr", bufs=2)
                nc.tensor.transpose(tp[:], src_ap, identR[:])
                ev_copy(dst_ap, tp[:])

            def make_xmT(xm):
                xmT = work.tile([P, TT, DC, P], F32R, tag="xmT")
                for t in range(TT):
                    for f in range(DC):
                        transpose_into(xmT[:, t, f, :], xm[:, t, f * P:(f + 1) * P])
                return xmT

            # ---------- layers ----------
            for l in range(L):
                bc = emit_bc(l * C6, 3 * D)          # sh_msa | sc_msa | g_msa
                if l == 0:
                    ddump("dbg_bc0", bc[:])
                n1b = pbcast_row(n1w_d[l, :, :])
                weff1 = weff(bc, D, n1b)
                xm = work.tile([P, TT, D], F32R, tag="xm")
                for t in range(TT):
                    ln_mod(t, weff1, bc[:, 0:D], xm)
                    if l == 0:
                        ddump("dbg_xm0", xm[:, t, :], t)
                xmT = make_xmT(xm)

                # qkv natural [128, t, 2304]; q rows pre-scaled by isqrt(hd)
                qkv = work.tile([P, TT, 3 * D], F32R, tag="bigA")
                for n in range(6):
                    pts = [ps.tile([P, 384], F32, tag="mm", bufs=3, name=f"qkvp{l}_{n}_{t}")
                           for t in range(TT)]
                    for f in range(DC):
                        wq = wts.tile([P, 384], F32R, tag="wts")
                        nc.sync.dma_start(wq[:], wqkv_d[l, n, f, :, :])
                        for t in range(TT):
                            nc.tensor.matmul(pts[t][:], xmT[:, t, f, :], wq[:],
                                             start=(f == 0), stop=(f == DC - 1))
                    for t in range(TT):
                        ev_copy(qkv[:, t, n * 384:(n + 1) * 384], pts[t][:])

                # rope on q,k in place (same tables; q pre-scaled on host)
                for t in range(TT):
                    for g in range(2):
                        base = g * D
                        seg = qkv[:, t, base:base + D].rearrange("p (h d) -> p h d", h=H)
                        rot = work.tile([P, D], F32, tag="rot")
                        rotv = rot[:, :].rearrange("p (h d) -> p h d", h=H)
                        nc.vector.tensor_scalar_mul(rotv[:, :, 0:HD // 2],
                                                    seg[:, :, HD // 2:HD], -1.0)
                        nc.vector.tensor_copy(rotv[:, :, HD // 2:HD], seg[:, :, 0:HD // 2])
                        t1 = work.tile([P, D], F32, tag="ropet")
                        nc.vector.tensor_tensor(t1[:], qkv[:, t, base:base + D],
                                                cos_sb[:, t, :], op=OP.mult)
                        nc.vector.tensor_tensor(rot[:], rot[:], sin_sb[:, t, :], op=OP.mult)
                        nc.vector.tensor_tensor(qkv[:, t, base:base + D], t1[:], rot[:],
                                                op=OP.add)
                    if l == 0:
                        ddump("dbg_qkv0", qkv[:, t, :], t)

                # local k^T (pre-AllGather transpose: 12 instead of 48)
                kTl = work.tile([P, DC, T], F32R, tag="kTl")
                for t in range(TT):
                    for f in range(DC):
                        transpose_into(kTl[:, f, t * P:(t + 1) * P],
                                       qkv[:, t, D + f * P:D + (f + 1) * P])
                if l == 0:
                    ddump("dbg_kTl0", kTl[:])

                # allgather kT | v within batch group (flat block per rank)
                kv_inf = dram.tile([DC * P * T + TT * P * D], F32R, tag="kvinf")
                for f in range(DC):
                    nc.sync.dma_start(
                        kv_inf[f * P * T:(f + 1) * P * T].rearrange("(p c) -> p c", p=P),
                        kTl[:, f, :])
                for t in range(TT):
                    nc.sync.dma_start(
                        kv_inf[KV1 + t * P * D:KV1 + (t + 1) * P * D]
                        .rearrange("(p c) -> p c", p=P),
                        qkv[:, t, 2 * D:3 * D])
                kv_out = dram.tile([TPD, DC * P * T + TT * P * D], F32R, tag="kvout")
                if CC_EMU:
                    for r in range(TPD):
                        nc.sync.dma_start(kv_out[r, :], kv_inf[:])
                else:
                    nc.gpsimd.collective_compute(
                        "AllGather", OP.bypass, replica_groups=GROUPS,
                        ins=[kv_inf[:].opt()], outs=[kv_out[:].opt()])

                # qT while AG is in flight
                qT = work.tile([P, DC, T], F32R, tag="qT")
                for t in range(TT):
                    for f in range(DC):
                        transpose_into(qT[:, f, t * P:(t + 1) * P],
                                       qkv[:, t, f * P:(f + 1) * P])
                if l == 0:
                    ddump("dbg_qT0", qT[:])

                # gathered k^T: [128, rank, f, 256];  v into v_aug
                agKT = work.tile([P, TPD, DC, T], F32R, tag="bigA")
                v_aug = work.tile([P, 2 * TPD, H, HD + 1], F32R, tag="bigB")
                nc.vector.tensor_copy(
                    v_aug[:, :, :, HD:HD + 1],
                    onesF[:, 0:1].unsqueeze(1).unsqueeze(1)
                    .to_broadcast([P, 2 * TPD, H, 1]))
                for r in range(TPD):
                    for f in range(DC):
                        nc.sync.dma_start(
                            agKT[:, r, f, :],
                            kv_out[r, f * P * T:(f + 1) * P * T]
                            .rearrange("(p c) -> p c", p=P))
                    for t in range(TT):
                        nc.sync.dma_start(
                            v_aug[:, r * TT + t, :, 0:HD],
                            kv_out[r, KV1 + t * P * D:KV1 + (t + 1) * P * D]
                            .rearrange("(p h d) -> p h d", p=P, h=H))
                if l == 0:
                    ddump("dbg_agKT0", agKT[:])
                    ddump("dbg_vaug0", v_aug[:])

                # attention (scores^T per head, no max subtraction)
                o_normT = work.tile([P, DC, T], F32R, tag="oT")
                for h in range(H):
                    po = (h % 2) * HD
                    fh = h // 2
                    av = ps.tile([HD + 1, T], F32, tag="av", bufs=1)
                    for c in range(8):
                        sp = ps.tile([P, T], F32, tag="mm", bufs=3)
                        nc.tensor.matmul(sp[:], agKT[po:po + HD, c // 2, fh,
                                                     (c % 2) * P:(c % 2 + 1) * P],
                                         qT[po:po + HD, fh, :], start=True, stop=True)
                        ec = work.tile([P, T], F32R, tag="expT", bufs=3)
                        nc.scalar.activation(ec[:], sp[:], AF.Exp)
                        if l == 0 and h == 0 and c == 0:
                            ddump("dbg_exp0", ec[:])
                        nc.tensor.matmul(av[:], v_aug[:, c, h, :], ec[:],
                                         start=(c == 0), stop=(c == 7))
                    rr = work.tile([1, T], F32, tag="rr", bufs=2)
                    nc.scalar.activation(rr[:], av[HD:HD + 1, :], AF.Copy)
                    rrc = work.tile([1, T], F32, tag="rrc", bufs=2)
                    nc.vector.reciprocal(rrc[:], rr[:])
                    rb = work.tile([HD, T], F32, tag="rb", bufs=2)
                    nc.gpsimd.partition_broadcast(rb[:], rrc[:])
                    nc.vector.tensor_tensor(o_normT[po:po + HD, fh, :],
                                            av[0:HD, :], rb[:], op=OP.mult)
                if l == 0:
                    ddump("dbg_onorm0", o_normT[:])

                # attn_out + gated residual
                for n in range(2):
                    pts = [ps.tile([P, 384], F32, tag="mm", bufs=3, name=f"aop{l}_{n}_{t}")
                           for t in range(TT)]
                    for f in range(DC):
                        wa = wts.tile([P, 384], F32R, tag="wts")
                        nc.sync.dma_start(wa[:], wao_d[l, n, f, :, :])
                        for t in range(TT):
                            nc.tensor.matmul(pts[t][:], o_normT[:, f, t * P:(t + 1) * P],
                                             wa[:], start=(f == 0), stop=(f == DC - 1))
                    for t in range(TT):
                        rt = work.tile([P, 384], F32, tag="rtmp", bufs=2)
                        nc.vector.tensor_tensor(rt[:], pts[t][:],
                                                bc[:, 2 * D + n * 384:2 * D + (n + 1) * 384],
                                                op=OP.mult)
                        nc.vector.tensor_tensor(x[:, t, n * 384:(n + 1) * 384],
                                                x[:, t, n * 384:(n + 1) * 384],
                                                rt[:], op=OP.add)
                if l == 0:
                    for t in range(TT):
                        ddump("dbg_xa0", x[:, t, :], t)

                # mlp: bc second half
                bc2 = emit_bc(l * C6 + 3 * D, 3 * D)  # sh_mlp | sc_mlp | g_mlp
                n2b = pbcast_row(n2w_d[l, :, :])
                weff2 = weff(bc2, D, n2b)
                xm2 = work.tile([P, TT, D], F32R, tag="xm")
                for t in range(TT):
                    ln_mod(t, weff2, bc2[:, 0:D], xm2)
                xm2T = make_xmT(xm2)

                # fused mlp1 -> gelu -> mlp2 accumulation, streamed per fc
                for half in range(2):
                    y2p = [ps.tile([P, 384], F32, tag="acc", bufs=2,
                                   name=f"y2p{l}_{half}_{t}") for t in range(TT)]
                    for fc in range(24):
                        w1t = wts.tile([P, DC, P], F32R, tag="wts")
                        nc.sync.dma_start(w1t[:], w1_d[l, fc, :, :, :])
                        hp = ps.tile([P, T], F32, tag="mm", bufs=3)
                        for d in range(DC):
                            nc.tensor.matmul(hp[:], w1t[:, d, :], xm2T[:, :, d, :],
                                             start=(d == 0), stop=(d == DC - 1))
                        hc = work.tile([P, T], F32R, tag="hc", bufs=3)
                        nc.scalar.activation(hc[:], hp[:], AF.Gelu_apprx_tanh,
                                             bias=b1T[:, l, fc:fc + 1])
                        w2t = wts.tile([P, 384], F32R, tag="wts")
                        nc.sync.dma_start(w2t[:], w2_d[l, fc * P:(fc + 1) * P,
                                                       half * 384:(half + 1) * 384])
                        for t in range(TT):
                            nc.tensor.matmul(y2p[t][:], hc[:, t * P:(t + 1) * P], w2t[:],
                                             start=(fc == 0), stop=False)
                    b2r = work.tile([1, 384], F32R, tag="b2r", bufs=2)
                    nc.sync.dma_start(b2r[:], b2_d[l, :, half * 384:(half + 1) * 384])
                    for t in range(TT):
                        nc.tensor.matmul(y2p[t][:], ones1r[:], b2r[:],
                                         start=False, stop=True)
                        rt = work.tile([P, 384], F32, tag="rtmp", bufs=2)
                        nc.vector.tensor_tensor(
                            rt[:], y2p[t][:],
                            bc2[:, 2 * D + half * 384:2 * D + (half + 1) * 384],
                            op=OP.mult)
                        nc.vector.tensor_tensor(x[:, t, half * 384:(half + 1) * 384],
                                                x[:, t, half * 384:(half + 1) * 384],
                                                rt[:], op=OP.add)
                if l == 0:
                    for t in range(TT):
                        ddump("dbg_x1", x[:, t, :], t)

            # ---------- final layer + unembed ----------
            bcf = emit_bc(L * C6, 2 * D)
            nfb = pbcast_row(nfw_d[:, :])
            wefff = weff(bcf, D, nfb)
            xmf = work.tile([P, TT, D], F32R, tag="xm")
            for t in range(TT):
                ln_mod(t, wefff, bcf[:, 0:D], xmf)
                ddump("dbg_xmf", xmf[:, t, :], t)
            xmfT = work.tile([P, DC, T], F32R, tag="qT")
            for t in range(TT):
                for f in range(DC):
                    transpose_into(xmfT[:, f, t * P:(t + 1) * P],
                                   xmf[:, t, f * P:(f + 1) * P])

            xf_in = dram.tile([DC, P, T], F32R, tag="xfin")
            for f in range(DC):
                nc.sync.dma_start(xf_in[f, :, :], xmfT[:, f, :])
            xf_out = dram.tile([TPD, DC, P, T], F32R, tag="xfout")
            if CC_EMU:
                for r in range(TPD):
                    nc.sync.dma_start(xf_out[r, :, :, :], xf_in[:, :, :])
            else:
                nc.gpsimd.collective_compute(
                    "AllGather", OP.bypass, replica_groups=GROUPS,
                    ins=[xf_in[:].opt()], outs=[xf_out[:].opt()])

            agX = work.tile([P, TPD, DC, T], F32R, tag="bigA")
            for r in range(TPD):
                for f in range(DC):
                    nc.sync.dma_start(agX[:, r, f, :], xf_out[r, f, :, :])
            ddump("dbg_agX", agX[:])

            for n in range(UN):
                lw = []
                for f in range(DC):
                    lwt = wts.tile([P, UNW], F32R, tag="lw", bufs=8, name=f"lw{n}_{f}")
                    nc.sync.dma_start(lwt[:], linwT_d[f * P:(f + 1) * P,
                                                      n * UNW:(n + 1) * UNW])
                    lw.append(lwt)
                lbr = work.tile([1, UNW], F32, tag="lbrow", bufs=2)
                nc.sync.dma_start(lbr[:], linb_d[:, n * UNW:(n + 1) * UNW])
                lb = work.tile([P, UNW], F32, tag="lb", bufs=2)
                nc.gpsimd.partition_broadcast(lb[:], lbr[:])
                for tc in range(8):
                    up = ps.tile([P, UNW], F32, tag="mm", bufs=3)
                    for f in range(DC):
                        nc.tensor.matmul(up[:],
                                         agX[:, tc // 2, f, (tc % 2) * P:(tc % 2 + 1) * P],
                                         lw[f][:], start=(f == 0), stop=(f == DC - 1))
                    ou = work.tile([P, UNW], F32, tag="osb", bufs=3)
                    nc.vector.tensor_tensor(ou[:], up[:], lb[:], op=OP.add)
                    nc.sync.dma_start(out_d[tc * P:(tc + 1) * P, n * UNW:(n + 1) * UNW],
                                      ou[:])

    nc.compile()
    return nc


_CACHE = {}


def _get_nc():
    if "nc" not in _CACHE:
        _CACHE["nc"] = _build()
    return _CACHE["nc"]


def _host_prep(inputs):
    g = {k: np.asarray(v) for k, v in inputs.items()}
    f32 = np.float32

    indices = g["indices"].astype(np.int32)          # [B, S]
    sigma = g["sigma"].astype(f32)                   # [B]
    qkv_w = g["qkv_w"].astype(f32).copy()            # [L, 3D, D]
    isd = f32(1.0 / math.sqrt(HD))
    qkv_w[:, 0:D, :] *= isd                          # fold 1/sqrt(hd) into q
    attn_out_w = g["attn_out_w"].astype(f32)
    mlp_w1 = g["mlp_w1"].astype(f32)
    mlp_w2 = g["mlp_w2"].astype(f32)
    adaLN_w = g["adaLN_w"].astype(f32)               # [L, 6D, COND]
    adaLN_b = g["adaLN_b"].astype(f32)
    adaLNf_w = g["adaLNf_w"].astype(f32)             # [2D, COND]
    adaLNf_b = g["adaLNf_b"].astype(f32)

    ac = np.ascontiguousarray

    wqkv = np.stack([qkv_w[l].T.reshape(DC, P, 6, 384).transpose(2, 0, 1, 3)
                     for l in range(L)])
    wao = np.stack([attn_out_w[l].T.reshape(DC, P, 2, 384).transpose(2, 0, 1, 3)
                    for l in range(L)])
    w1 = np.stack([mlp_w1[l].T.reshape(DC, P, 24, P).transpose(2, 1, 0, 3)
                   for l in range(L)])
    b1T = np.stack([g["mlp_b1"].astype(f32)[l].reshape(24, P).T for l in range(L)])
    w2 = np.stack([mlp_w2[l].T for l in range(L)])
    b2r = g["mlp_b2"].astype(f32).reshape(L, 1, D)

    adawT = np.concatenate(
        [adaLN_w[l].T for l in range(L)] + [adaLNf_w.T], axis=1)  # [128, 19968]
    badd = np.concatenate([adaLN_b.reshape(-1), adaLNf_b]).copy()
    for l in range(L):
        badd[l * C6 + D:l * C6 + 2 * D] += 1.0       # sc_msa
        badd[l * C6 + 4 * D:l * C6 + 5 * D] += 1.0   # sc_mlp
    badd[L * C6 + D:L * C6 + 2 * D] += 1.0           # final sc
    badd = badd.reshape(1, -1)

    half = FREQ // 2
    freqsT = np.exp(-math.log(10000.0) * np.arange(half, dtype=f32) / half).reshape(P, 1)
    tw1T = g["tmlp_w1"].astype(f32).T.reshape(2, P, P)
    tw2T = g["tmlp_w2"].astype(f32).T
    tb1 = g["tmlp_b1"].astype(f32).reshape(1, P)
    tb2 = g["tmlp_b2"].astype(f32).reshape(1, P)

    inv_freq = 1.0 / (10000.0 ** (np.arange(0, HD, 2, dtype=f32) / HD))
    fr = np.arange(S, dtype=f32)[:, None] * inv_freq[None, :]
    emb = np.concatenate([fr, fr], axis=-1)          # [S, HD]
    cosf = np.cos(emb).astype(f32)
    sinf = np.sin(emb).astype(f32)

    n1w = g["norm1_w"].astype(f32).reshape(L, 1, D)
    n2w = g["norm2_w"].astype(f32).reshape(L, 1, D)
    nfw = g["normf_w"].astype(f32).reshape(1, D)
    embtab = g["embedding"].astype(f32)
    linwT = g["lin_w"].astype(f32).T                 # [768, 32000]
    lin_b = g["lin_b"].astype(f32)

    shared = dict(
        embtab=ac(embtab), freqsT=ac(freqsT), tw1T=ac(tw1T), tb1=ac(tb1),
        tw2T=ac(tw2T), tb2=ac(tb2), adawT=ac(adawT.astype(f32)),
        badd=ac(badd.astype(f32)), n1w=ac(n1w), n2w=ac(n2w), nfw=ac(nfw),
        wqkv=ac(wqkv), wao=ac(wao), w1=ac(w1), b1T=ac(b1T), w2=ac(w2), b2r=ac(b2r),
    )

    in_maps = []
    for c in range(NCORES):
        b, r = c // TPD, c % TPD
        tok = slice(r * T, (r + 1) * T)
        m = dict(shared)
        m["idx"] = ac(indices[b, tok].reshape(T, 1))
        m["sig11"] = ac(sigma[b].reshape(1, 1))
        m["cosn"] = ac(np.tile(cosf[tok], (1, H)))
        m["sinn"] = ac(np.tile(sinf[tok], (1, H)))
        m["linwT"] = ac(linwT[:, r * VSH:(r + 1) * VSH])
        m["linb"] = ac(lin_b[r * VSH:(r + 1) * VSH].reshape(1, VSH))
        in_maps.append(m)
    return in_maps


def run(inputs, trace=False):
    nc = _get_nc()
    in_maps = _host_prep(inputs)
    res = run_bass_kernel_spmd(nc, in_maps, core_ids=list(range(NCORES)),
                               trace=trace)
    out = np.empty((B, S, V), dtype=np.float32)
    for c in range(NCORES):
        b, r = c // TPD, c % TPD
        shard = res.results[c]["out"]                # [1024, 8000]
        out[b, :, r * VSH:(r + 1) * VSH] = shard
    return out, res


def kernel(**inputs) -> np.ndarray:
    return run(inputs, trace=False)[0]

